# revision 8
# baseline (speedup 1.0000x reference)
"""MinGRU on Trainium2 (Bass/Tile), data-parallel over batch on 8 NeuronCores.

Math (per batch element, per hidden channel):
    k_z = x @ W_z.T + b_z
    k_h = x @ W_h.T + b_h
    a   = sigmoid(-k_z)                  # = exp(log_coeffs) in the reference
    z   = sigmoid(k_z) = 1 - a
    g(u)= u + 0.5 if u >= 0 else sigmoid(u)
    v   = z * g(k_h)                     # = exp(log_values[1:])
    h_t = a_t * h_{t-1} + v_t,  h_init = g(h_0)        (t = 1..T)
Output is h_1..h_T, shape [B, T, H].

Device layout: each core gets one batch element. Hidden dim H lives on
SBUF partitions (8 tiles of 128), time T on the free dim, so the
recurrence maps to the DVE TensorTensorScan instruction:
    state = (a * state) - w      with  w = (a - 1) * g = -v
Matmuls run as out[H_tile, T_chunk] = W.T[I_tile, H_tile].T @ x.T[I_tile, T_chunk]
with fp32r operands (fp32 storage, full PE rate at N=512).

Host side pre-transposes x -> x.T and W -> W.T (layout prep only) and
transposes the [H, T] per-core output back to [T, H].
"""

import numpy as np
from contextlib import ExitStack

import concourse.bass as bass
import concourse.tile as tile
from concourse import bacc, mybir
from concourse.bass_utils import run_bass_kernel_spmd

B, T, I, H = 8, 4096, 1024, 1024
P = 128           # SBUF partitions
TC = 1024         # max T chunk for the PSUM/ACT/DVE stage (2 PSUM banks)
MN = 512          # matmul moving free dim (one PSUM bank of fp32)
# Tapered chunk schedule: big chunks amortize per-op overhead; small final
# chunks shrink the post-matmul tail (ACT->DVE->scan chain after PE ends).
CHUNKS = [1024, 1024, 1024, 512, 512]
assert sum(CHUNKS) == T
NI, NH = I // P, H // P
NT = len(CHUNKS)
F32 = mybir.dt.float32
F32R = mybir.dt.float32r
AF = mybir.ActivationFunctionType
OP = mybir.AluOpType

_PROGRAM = None


def _build_program():
    nc = bacc.Bacc("TRN2", target_bir_lowering=False, debug=False)
    xT = nc.dram_tensor("xT", [I, T], F32R, kind="ExternalInput").ap()
    wzT = nc.dram_tensor("wzT", [I, H], F32R, kind="ExternalInput").ap()
    whT = nc.dram_tensor("whT", [I, H], F32R, kind="ExternalInput").ap()
    nbz = nc.dram_tensor("nbz", [H], F32, kind="ExternalInput").ap()   # -b_z
    bh = nc.dram_tensor("bh", [H], F32, kind="ExternalInput").ap()
    h0 = nc.dram_tensor("h0", [H], F32, kind="ExternalInput").ap()
    out = nc.dram_tensor("out", [H, T], F32, kind="ExternalOutput").ap()

    with tile.TileContext(nc) as tc, ExitStack() as ctx:
        const = ctx.enter_context(tc.tile_pool(name="const", bufs=1))
        xpool = ctx.enter_context(tc.tile_pool(name="xp", bufs=2))
        psum = ctx.enter_context(tc.tile_pool(name="ps", bufs=2, space="PSUM"))
        act = ctx.enter_context(tc.tile_pool(name="actp", bufs=2))
        hpool = ctx.enter_context(tc.tile_pool(name="hp", bufs=3))

        wzT_r = wzT.rearrange("(n p) h -> p n h", p=P)
        whT_r = whT.rearrange("(n p) h -> p n h", p=P)
        xT_r = xT.rearrange("(n p) t -> p n t", p=P)

        # x chunk 0 on the SP HWDGE ring, weights on the ACT HWDGE ring —
        # the two rings drain in parallel, so the first matmul starts after
        # ~1 MB of DMA instead of 10 MB.
        x_tiles = [[None] * NI for _ in range(NT)]
        wz_sb, wh_sb = [], []
        for i in range(NI):
            x0_i = xpool.tile([P, CHUNKS[0]], F32R, tag=f"x{i}", name=f"x_0_{i}")
            nc.sync.dma_start(x0_i[:], xT_r[:, i, 0:CHUNKS[0]])
            x_tiles[0][i] = x0_i
            wz_i = const.tile([P, H], F32R, tag=f"wz{i}", name=f"wz_sb{i}")
            wh_i = const.tile([P, H], F32R, tag=f"wh{i}", name=f"wh_sb{i}")
            nc.scalar.dma_start(wz_i[:], wzT_r[:, i, :])
            nc.scalar.dma_start(wh_i[:], whT_r[:, i, :])
            wz_sb.append(wz_i)
            wh_sb.append(wh_i)

        # small constants (consumed only after the first 16 matmuls)
        nbz_sb = const.tile([P, NH], F32, tag="nbz", name="nbz_sb")
        bh_sb = const.tile([P, NH], F32, tag="bh", name="bh_sb")
        h0_sb = const.tile([P, NH], F32, tag="h0", name="h0_sb")
        nc.scalar.dma_start(nbz_sb[:], nbz.rearrange("(n p) -> p n", p=P))
        nc.scalar.dma_start(bh_sb[:], bh.rearrange("(n p) -> p n", p=P))
        nc.scalar.dma_start(h0_sb[:], h0.rearrange("(n p) -> p n", p=P))

        # g(h_0) -> scan carry [P, NH]; carry[:, j] always holds the last
        # hidden state of channel block j.
        s0 = const.tile([P, NH], F32, tag="s0", name="s0")
        r0 = const.tile([P, NH], F32, tag="r0", name="r0")
        carry = const.tile([P, NH], F32, tag="carry", name="carry")
        nc.scalar.activation(s0[:], h0_sb[:], AF.Sigmoid)
        nc.scalar.activation(r0[:], h0_sb[:], AF.Relu)
        nc.vector.scalar_tensor_tensor(
            carry[:], s0[:], 0.5, r0[:], op0=OP.min, op1=OP.add
        )

        off = 0
        for t, tcn in enumerate(CHUNKS):
            if t + 1 < NT:
                noff = off + tcn
                for i in range(NI):
                    xn_i = xpool.tile(
                        [P, CHUNKS[t + 1]], F32R, tag=f"x{i}", name=f"x_{t + 1}_{i}"
                    )
                    nc.sync.dma_start(
                        xn_i[:], xT_r[:, i, noff:noff + CHUNKS[t + 1]]
                    )
                    x_tiles[t + 1][i] = xn_i
            for j in range(NH):
                kz = psum.tile([P, tcn], F32, tag="kz", name=f"kz_{t}_{j}")
                kh = psum.tile([P, tcn], F32, tag="kh", name=f"kh_{t}_{j}")
                for ps, w_sb in ((kz, wz_sb), (kh, wh_sb)):
                    for i in range(NI):
                        for half in range(tcn // MN):
                            nc.tensor.matmul(
                                ps[:, half * MN:(half + 1) * MN],
                                w_sb[i][:, j * P:(j + 1) * P],
                                x_tiles[t][i][:, half * MN:(half + 1) * MN],
                                start=(i == 0),
                                stop=(i == NI - 1),
                            )
                a_t = act.tile([P, tcn], F32, tag="a", name=f"a_{t}_{j}")
                s_t = act.tile([P, tcn], F32, tag="s", name=f"s_{t}_{j}")
                r_t = act.tile([P, tcn], F32, tag="r", name=f"r_{t}_{j}")
                g_t = act.tile([P, tcn], F32, tag="g", name=f"g_{t}_{j}")
                w_t = act.tile([P, tcn], F32, tag="w", name=f"w_{t}_{j}")
                nc.scalar.activation(
                    a_t[:], kz[:], AF.Sigmoid, bias=nbz_sb[:, j:j + 1], scale=-1.0
                )
                nc.scalar.activation(
                    s_t[:], kh[:], AF.Sigmoid, bias=bh_sb[:, j:j + 1], scale=1.0
                )
                nc.scalar.activation(
                    r_t[:], kh[:], AF.Relu, bias=bh_sb[:, j:j + 1], scale=1.0
                )
                nc.vector.scalar_tensor_tensor(
                    g_t[:], s_t[:], 0.5, r_t[:], op0=OP.min, op1=OP.add
                )
                nc.vector.scalar_tensor_tensor(
                    w_t[:], a_t[:], 1.0, g_t[:], op0=OP.subtract, op1=OP.mult
                )
                h_t = hpool.tile([P, tcn], F32, tag="h", name=f"h_{t}_{j}")
                nc.vector.tensor_tensor_scan(
                    h_t[:], a_t[:], w_t[:], carry[:, j:j + 1],
                    op0=OP.mult, op1=OP.subtract
                )
                nc.scalar.copy(carry[:, j:j + 1], h_t[:, tcn - 1:tcn])
                nc.sync.dma_start(
                    out[j * P:(j + 1) * P, off:off + tcn], h_t[:]
                )
            off += tcn

    nc.compile()
    return nc


def _get_program():
    global _PROGRAM
    if _PROGRAM is None:
        _PROGRAM = _build_program()
    return _PROGRAM


def _make_in_maps(x, h_0, W_z, b_z, W_h, b_h):
    wzT = np.ascontiguousarray(W_z.T.astype(np.float32))
    whT = np.ascontiguousarray(W_h.T.astype(np.float32))
    nbz = np.ascontiguousarray(-b_z.astype(np.float32))
    bh = np.ascontiguousarray(b_h.astype(np.float32))
    in_maps = []
    for b in range(B):
        in_maps.append({
            "xT": np.ascontiguousarray(x[b].T.astype(np.float32)),
            "wzT": wzT,
            "whT": whT,
            "nbz": nbz,
            "bh": bh,
            "h0": np.ascontiguousarray(h_0[b].astype(np.float32)),
        })
    return in_maps


def _run(x, h_0, W_z, b_z, W_h, b_h, trace=False):
    nc = _get_program()
    in_maps = _make_in_maps(x, h_0, W_z, b_z, W_h, b_h)
    res = run_bass_kernel_spmd(nc, in_maps, core_ids=list(range(B)), trace=trace)
    out = np.stack(
        [res.results[b]["out"].T for b in range(B)], axis=0
    ).astype(np.float32)
    return out, res


def kernel(x, h_0, W_z, b_z, W_h, b_h):
    out, _ = _run(x, h_0, W_z, b_z, W_h, b_h)
    return out


# revision 11
# speedup vs baseline: 1.0054x; 1.0054x over previous
"""MinGRU on Trainium2 (Bass/Tile), data-parallel over batch on 8 NeuronCores.

Math (per batch element, per hidden channel):
    k_z = x @ W_z.T + b_z
    k_h = x @ W_h.T + b_h
    a   = sigmoid(-k_z)                  # = exp(log_coeffs) in the reference
    z   = sigmoid(k_z) = 1 - a
    g(u)= u + 0.5 if u >= 0 else sigmoid(u)
    v   = z * g(k_h)                     # = exp(log_values[1:])
    h_t = a_t * h_{t-1} + v_t,  h_init = g(h_0)        (t = 1..T)
Output is h_1..h_T, shape [B, T, H].

Device layout: each core gets one batch element. Hidden dim H lives on
SBUF partitions (8 tiles of 128), time T on the free dim, so the
recurrence maps to the DVE TensorTensorScan instruction:
    state = (a * state) - w      with  w = (a - 1) * g = -v
Matmuls run as out[H_tile, T_chunk] = W.T[I_tile, H_tile].T @ x.T[I_tile, T_chunk]
with fp32r operands (fp32 storage, full PE rate at N=512).

Host side pre-transposes x -> x.T and W -> W.T (layout prep only) and
transposes the [H, T] per-core output back to [T, H].
"""

import numpy as np
from contextlib import ExitStack

import concourse.bass as bass
import concourse.tile as tile
from concourse import bacc, mybir
from concourse.bass_utils import run_bass_kernel_spmd

B, T, I, H = 8, 4096, 1024, 1024
P = 128           # SBUF partitions
TC = 1024         # max T chunk for the PSUM/ACT/DVE stage (2 PSUM banks)
MN = 512          # matmul moving free dim (one PSUM bank of fp32)
# Tapered chunk schedule: big chunks amortize per-op overhead; small final
# chunks shrink the post-matmul tail (ACT->DVE->scan chain after PE ends).
CHUNKS = [512, 1024, 1024, 1024, 512]
assert sum(CHUNKS) == T
NI, NH = I // P, H // P
NT = len(CHUNKS)
F32 = mybir.dt.float32
F32R = mybir.dt.float32r
AF = mybir.ActivationFunctionType
OP = mybir.AluOpType

_PROGRAM = None


def _build_program():
    nc = bacc.Bacc("TRN2", target_bir_lowering=False, debug=False)
    xT = nc.dram_tensor("xT", [I, T], F32R, kind="ExternalInput").ap()
    wzT = nc.dram_tensor("wzT", [I, H], F32R, kind="ExternalInput").ap()
    whT = nc.dram_tensor("whT", [I, H], F32R, kind="ExternalInput").ap()
    nbz = nc.dram_tensor("nbz", [H], F32, kind="ExternalInput").ap()   # -b_z
    bh = nc.dram_tensor("bh", [H], F32, kind="ExternalInput").ap()
    h0 = nc.dram_tensor("h0", [H], F32, kind="ExternalInput").ap()
    out = nc.dram_tensor("out", [H, T], F32, kind="ExternalOutput").ap()

    with tile.TileContext(nc) as tc, ExitStack() as ctx:
        const = ctx.enter_context(tc.tile_pool(name="const", bufs=1))
        xpool = ctx.enter_context(tc.tile_pool(name="xp", bufs=2))
        psum = ctx.enter_context(tc.tile_pool(name="ps", bufs=2, space="PSUM"))
        act = ctx.enter_context(tc.tile_pool(name="actp", bufs=2))
        hpool = ctx.enter_context(tc.tile_pool(name="hp", bufs=3))

        wzT_r = wzT.rearrange("(n p) h -> p n h", p=P)
        whT_r = whT.rearrange("(n p) h -> p n h", p=P)
        xT_r = xT.rearrange("(n p) t -> p n t", p=P)

        # x chunk 0 on the SP HWDGE ring, weights on the ACT HWDGE ring —
        # the two rings drain in parallel, so the first matmul starts after
        # ~1 MB of DMA instead of 10 MB.
        x_tiles = [[None] * NI for _ in range(NT)]
        wz_sb, wh_sb = [], []
        for i in range(NI):
            x0_i = xpool.tile([P, CHUNKS[0]], F32R, tag=f"x{i}", name=f"x_0_{i}")
            nc.sync.dma_start(x0_i[:], xT_r[:, i, 0:CHUNKS[0]])
            x_tiles[0][i] = x0_i
            wz_i = const.tile([P, H], F32R, tag=f"wz{i}", name=f"wz_sb{i}")
            wh_i = const.tile([P, H], F32R, tag=f"wh{i}", name=f"wh_sb{i}")
            nc.gpsimd.dma_start(wz_i[:], wzT_r[:, i, :])
            nc.gpsimd.dma_start(wh_i[:], whT_r[:, i, :])
            wz_sb.append(wz_i)
            wh_sb.append(wh_i)

        # small constants (consumed only after the first 16 matmuls)
        nbz_sb = const.tile([P, NH], F32, tag="nbz", name="nbz_sb")
        bh_sb = const.tile([P, NH], F32, tag="bh", name="bh_sb")
        h0_sb = const.tile([P, NH], F32, tag="h0", name="h0_sb")
        nc.scalar.dma_start(nbz_sb[:], nbz.rearrange("(n p) -> p n", p=P))
        nc.scalar.dma_start(bh_sb[:], bh.rearrange("(n p) -> p n", p=P))
        nc.scalar.dma_start(h0_sb[:], h0.rearrange("(n p) -> p n", p=P))

        # g(h_0) -> scan carry [P, NH]; carry[:, j] always holds the last
        # hidden state of channel block j.
        s0 = const.tile([P, NH], F32, tag="s0", name="s0")
        r0 = const.tile([P, NH], F32, tag="r0", name="r0")
        carry = const.tile([P, NH], F32, tag="carry", name="carry")
        nc.scalar.activation(s0[:], h0_sb[:], AF.Sigmoid)
        nc.scalar.activation(r0[:], h0_sb[:], AF.Relu)
        nc.vector.scalar_tensor_tensor(
            carry[:], s0[:], 0.5, r0[:], op0=OP.min, op1=OP.add
        )

        off = 0
        for t, tcn in enumerate(CHUNKS):
            if t + 1 < NT:
                noff = off + tcn
                for i in range(NI):
                    xn_i = xpool.tile(
                        [P, CHUNKS[t + 1]], F32R, tag=f"x{i}", name=f"x_{t + 1}_{i}"
                    )
                    nc.sync.dma_start(
                        xn_i[:], xT_r[:, i, noff:noff + CHUNKS[t + 1]]
                    )
                    x_tiles[t + 1][i] = xn_i
            for j in range(NH):
                kz = psum.tile([P, tcn], F32, tag="kz", name=f"kz_{t}_{j}")
                kh = psum.tile([P, tcn], F32, tag="kh", name=f"kh_{t}_{j}")
                for ps, w_sb in ((kz, wz_sb), (kh, wh_sb)):
                    for i in range(NI):
                        for half in range(tcn // MN):
                            nc.tensor.matmul(
                                ps[:, half * MN:(half + 1) * MN],
                                w_sb[i][:, j * P:(j + 1) * P],
                                x_tiles[t][i][:, half * MN:(half + 1) * MN],
                                start=(i == 0),
                                stop=(i == NI - 1),
                            )
                a_t = act.tile([P, tcn], F32, tag="a", name=f"a_{t}_{j}")
                s_t = act.tile([P, tcn], F32, tag="s", name=f"s_{t}_{j}")
                r_t = act.tile([P, tcn], F32, tag="r", name=f"r_{t}_{j}")
                g_t = act.tile([P, tcn], F32, tag="g", name=f"g_{t}_{j}")
                w_t = act.tile([P, tcn], F32, tag="w", name=f"w_{t}_{j}")
                nc.scalar.activation(
                    a_t[:], kz[:], AF.Sigmoid, bias=nbz_sb[:, j:j + 1], scale=-1.0
                )
                nc.scalar.activation(
                    s_t[:], kh[:], AF.Sigmoid, bias=bh_sb[:, j:j + 1], scale=1.0
                )
                nc.scalar.activation(
                    r_t[:], kh[:], AF.Relu, bias=bh_sb[:, j:j + 1], scale=1.0
                )
                nc.vector.scalar_tensor_tensor(
                    g_t[:], s_t[:], 0.5, r_t[:], op0=OP.min, op1=OP.add
                )
                nc.vector.scalar_tensor_tensor(
                    w_t[:], a_t[:], 1.0, g_t[:], op0=OP.subtract, op1=OP.mult
                )
                h_t = hpool.tile([P, tcn], F32, tag="h", name=f"h_{t}_{j}")
                nc.vector.tensor_tensor_scan(
                    h_t[:], a_t[:], w_t[:], carry[:, j:j + 1],
                    op0=OP.mult, op1=OP.subtract
                )
                nc.scalar.copy(carry[:, j:j + 1], h_t[:, tcn - 1:tcn])
                nc.sync.dma_start(
                    out[j * P:(j + 1) * P, off:off + tcn], h_t[:]
                )
            off += tcn

    nc.compile()
    return nc


def _get_program():
    global _PROGRAM
    if _PROGRAM is None:
        _PROGRAM = _build_program()
    return _PROGRAM


def _make_in_maps(x, h_0, W_z, b_z, W_h, b_h):
    wzT = np.ascontiguousarray(W_z.T.astype(np.float32))
    whT = np.ascontiguousarray(W_h.T.astype(np.float32))
    nbz = np.ascontiguousarray(-b_z.astype(np.float32))
    bh = np.ascontiguousarray(b_h.astype(np.float32))
    in_maps = []
    for b in range(B):
        in_maps.append({
            "xT": np.ascontiguousarray(x[b].T.astype(np.float32)),
            "wzT": wzT,
            "whT": whT,
            "nbz": nbz,
            "bh": bh,
            "h0": np.ascontiguousarray(h_0[b].astype(np.float32)),
        })
    return in_maps


def _run(x, h_0, W_z, b_z, W_h, b_h, trace=False):
    nc = _get_program()
    in_maps = _make_in_maps(x, h_0, W_z, b_z, W_h, b_h)
    res = run_bass_kernel_spmd(nc, in_maps, core_ids=list(range(B)), trace=trace)
    out = np.stack(
        [res.results[b]["out"].T for b in range(B)], axis=0
    ).astype(np.float32)
    return out, res


def kernel(x, h_0, W_z, b_z, W_h, b_h):
    out, _ = _run(x, h_0, W_z, b_z, W_h, b_h)
    return out


# revision 12
# speedup vs baseline: 1.0840x; 1.0782x over previous
"""MinGRU on Trainium2 (Bass/Tile), data-parallel over batch on 8 NeuronCores.

Math (per batch element, per hidden channel):
    k_z = x @ W_z.T + b_z
    k_h = x @ W_h.T + b_h
    a   = sigmoid(-k_z)                  # = exp(log_coeffs) in the reference
    z   = sigmoid(k_z) = 1 - a
    g(u)= u + 0.5 if u >= 0 else sigmoid(u)
    v   = z * g(k_h)                     # = exp(log_values[1:])
    h_t = a_t * h_{t-1} + v_t,  h_init = g(h_0)        (t = 1..T)
Output is h_1..h_T, shape [B, T, H].

Device layout: each core gets one batch element. Hidden dim H lives on
SBUF partitions (8 tiles of 128), time T on the free dim, so the
recurrence maps to the DVE TensorTensorScan instruction:
    state = (a * state) - w      with  w = (a - 1) * g = -v
Matmuls run as out[H_tile, T_chunk] = W.T[I_tile, H_tile].T @ x.T[I_tile, T_chunk]
with fp32r operands (fp32 storage, full PE rate at N=512).

Host side pre-transposes x -> x.T and W -> W.T (layout prep only) and
transposes the [H, T] per-core output back to [T, H].
"""

import numpy as np
from contextlib import ExitStack

import concourse.bass as bass
import concourse.tile as tile
from concourse import bacc, mybir
from concourse.bass_utils import run_bass_kernel_spmd

B, T, I, H = 8, 4096, 1024, 1024
P = 128           # SBUF partitions
TC = 1024         # max T chunk for the PSUM/ACT/DVE stage (2 PSUM banks)
MN = 512          # matmul moving free dim (one PSUM bank of fp32)
# Tapered chunk schedule: big chunks amortize per-op overhead; small final
# chunks shrink the post-matmul tail (ACT->DVE->scan chain after PE ends).
CHUNKS = [512, 1024, 1024, 1024, 512]
assert sum(CHUNKS) == T
NI, NH = I // P, H // P
NT = len(CHUNKS)
F32 = mybir.dt.float32
F32R = mybir.dt.float32r
BF16 = mybir.dt.bfloat16
MM_DT = BF16          # matmul operand dtype: F32R (tf32-ish) or BF16
import ml_dtypes
MM_NP = ml_dtypes.bfloat16 if MM_DT == BF16 else np.float32
AF = mybir.ActivationFunctionType
OP = mybir.AluOpType

_PROGRAM = None


def _build_program():
    nc = bacc.Bacc("TRN2", target_bir_lowering=False, debug=False)
    xT = nc.dram_tensor("xT", [I, T], MM_DT, kind="ExternalInput").ap()
    wzT = nc.dram_tensor("wzT", [I, H], MM_DT, kind="ExternalInput").ap()
    whT = nc.dram_tensor("whT", [I, H], MM_DT, kind="ExternalInput").ap()
    nbz = nc.dram_tensor("nbz", [H], F32, kind="ExternalInput").ap()   # -b_z
    bh = nc.dram_tensor("bh", [H], F32, kind="ExternalInput").ap()
    h0 = nc.dram_tensor("h0", [H], F32, kind="ExternalInput").ap()
    out = nc.dram_tensor("out", [H, T], F32, kind="ExternalOutput").ap()

    with tile.TileContext(nc) as tc, ExitStack() as ctx:
        const = ctx.enter_context(tc.tile_pool(name="const", bufs=1))
        xpool = ctx.enter_context(tc.tile_pool(name="xp", bufs=2))
        psum = ctx.enter_context(tc.tile_pool(name="ps", bufs=2, space="PSUM"))
        act = ctx.enter_context(tc.tile_pool(name="actp", bufs=2))
        hpool = ctx.enter_context(tc.tile_pool(name="hp", bufs=3))

        wzT_r = wzT.rearrange("(n p) h -> p n h", p=P)
        whT_r = whT.rearrange("(n p) h -> p n h", p=P)
        xT_r = xT.rearrange("(n p) t -> p n t", p=P)

        # x chunk 0 on the SP HWDGE ring, weights on the ACT HWDGE ring —
        # the two rings drain in parallel, so the first matmul starts after
        # ~1 MB of DMA instead of 10 MB.
        x_tiles = [[None] * NI for _ in range(NT)]
        wz_sb, wh_sb = [], []
        for i in range(NI):
            x0_i = xpool.tile([P, CHUNKS[0]], MM_DT, tag=f"x{i}", name=f"x_0_{i}")
            nc.sync.dma_start(x0_i[:], xT_r[:, i, 0:CHUNKS[0]])
            x_tiles[0][i] = x0_i
            wz_i = const.tile([P, H], MM_DT, tag=f"wz{i}", name=f"wz_sb{i}")
            wh_i = const.tile([P, H], MM_DT, tag=f"wh{i}", name=f"wh_sb{i}")
            nc.gpsimd.dma_start(wz_i[:], wzT_r[:, i, :])
            nc.gpsimd.dma_start(wh_i[:], whT_r[:, i, :])
            wz_sb.append(wz_i)
            wh_sb.append(wh_i)

        # small constants (consumed only after the first 16 matmuls)
        nbz_sb = const.tile([P, NH], F32, tag="nbz", name="nbz_sb")
        bh_sb = const.tile([P, NH], F32, tag="bh", name="bh_sb")
        h0_sb = const.tile([P, NH], F32, tag="h0", name="h0_sb")
        nc.scalar.dma_start(nbz_sb[:], nbz.rearrange("(n p) -> p n", p=P))
        nc.scalar.dma_start(bh_sb[:], bh.rearrange("(n p) -> p n", p=P))
        nc.scalar.dma_start(h0_sb[:], h0.rearrange("(n p) -> p n", p=P))

        # g(h_0) -> scan carry [P, NH]; carry[:, j] always holds the last
        # hidden state of channel block j.
        s0 = const.tile([P, NH], F32, tag="s0", name="s0")
        r0 = const.tile([P, NH], F32, tag="r0", name="r0")
        carry = const.tile([P, NH], F32, tag="carry", name="carry")
        nc.scalar.activation(s0[:], h0_sb[:], AF.Sigmoid)
        nc.scalar.activation(r0[:], h0_sb[:], AF.Relu)
        nc.vector.scalar_tensor_tensor(
            carry[:], s0[:], 0.5, r0[:], op0=OP.min, op1=OP.add
        )

        off = 0
        for t, tcn in enumerate(CHUNKS):
            if t + 1 < NT:
                noff = off + tcn
                for i in range(NI):
                    xn_i = xpool.tile(
                        [P, CHUNKS[t + 1]], MM_DT, tag=f"x{i}", name=f"x_{t + 1}_{i}"
                    )
                    nc.sync.dma_start(
                        xn_i[:], xT_r[:, i, noff:noff + CHUNKS[t + 1]]
                    )
                    x_tiles[t + 1][i] = xn_i
            for j in range(NH):
                kz = psum.tile([P, tcn], F32, tag="kz", name=f"kz_{t}_{j}")
                kh = psum.tile([P, tcn], F32, tag="kh", name=f"kh_{t}_{j}")
                for ps, w_sb in ((kz, wz_sb), (kh, wh_sb)):
                    for i in range(NI):
                        for half in range(tcn // MN):
                            nc.tensor.matmul(
                                ps[:, half * MN:(half + 1) * MN],
                                w_sb[i][:, j * P:(j + 1) * P],
                                x_tiles[t][i][:, half * MN:(half + 1) * MN],
                                start=(i == 0),
                                stop=(i == NI - 1),
                            )
                a_t = act.tile([P, tcn], F32, tag="a", name=f"a_{t}_{j}")
                s_t = act.tile([P, tcn], F32, tag="s", name=f"s_{t}_{j}")
                r_t = act.tile([P, tcn], F32, tag="r", name=f"r_{t}_{j}")
                g_t = act.tile([P, tcn], F32, tag="g", name=f"g_{t}_{j}")
                w_t = act.tile([P, tcn], F32, tag="w", name=f"w_{t}_{j}")
                nc.scalar.activation(
                    a_t[:], kz[:], AF.Sigmoid, bias=nbz_sb[:, j:j + 1], scale=-1.0
                )
                nc.scalar.activation(
                    s_t[:], kh[:], AF.Sigmoid, bias=bh_sb[:, j:j + 1], scale=1.0
                )
                nc.scalar.activation(
                    r_t[:], kh[:], AF.Relu, bias=bh_sb[:, j:j + 1], scale=1.0
                )
                nc.vector.scalar_tensor_tensor(
                    g_t[:], s_t[:], 0.5, r_t[:], op0=OP.min, op1=OP.add
                )
                nc.vector.scalar_tensor_tensor(
                    w_t[:], a_t[:], 1.0, g_t[:], op0=OP.subtract, op1=OP.mult
                )
                h_t = hpool.tile([P, tcn], F32, tag="h", name=f"h_{t}_{j}")
                nc.vector.tensor_tensor_scan(
                    h_t[:], a_t[:], w_t[:], carry[:, j:j + 1],
                    op0=OP.mult, op1=OP.subtract
                )
                nc.scalar.copy(carry[:, j:j + 1], h_t[:, tcn - 1:tcn])
                nc.sync.dma_start(
                    out[j * P:(j + 1) * P, off:off + tcn], h_t[:]
                )
            off += tcn

    nc.compile()
    return nc


def _get_program():
    global _PROGRAM
    if _PROGRAM is None:
        _PROGRAM = _build_program()
    return _PROGRAM


def _make_in_maps(x, h_0, W_z, b_z, W_h, b_h):
    wzT = np.ascontiguousarray(W_z.T.astype(MM_NP))
    whT = np.ascontiguousarray(W_h.T.astype(MM_NP))
    nbz = np.ascontiguousarray(-b_z.astype(np.float32))
    bh = np.ascontiguousarray(b_h.astype(np.float32))
    in_maps = []
    for b in range(B):
        in_maps.append({
            "xT": np.ascontiguousarray(x[b].T.astype(MM_NP)),
            "wzT": wzT,
            "whT": whT,
            "nbz": nbz,
            "bh": bh,
            "h0": np.ascontiguousarray(h_0[b].astype(np.float32)),
        })
    return in_maps


def _run(x, h_0, W_z, b_z, W_h, b_h, trace=False):
    nc = _get_program()
    in_maps = _make_in_maps(x, h_0, W_z, b_z, W_h, b_h)
    res = run_bass_kernel_spmd(nc, in_maps, core_ids=list(range(B)), trace=trace)
    out = np.stack(
        [res.results[b]["out"].T for b in range(B)], axis=0
    ).astype(np.float32)
    return out, res


def kernel(x, h_0, W_z, b_z, W_h, b_h):
    out, _ = _run(x, h_0, W_z, b_z, W_h, b_h)
    return out
